# revision 1
# baseline (speedup 1.0000x reference)
"""Trainium2 Bass kernel for a CPC-style loss (graph pooling + NCE + distance).

Strategy (8 NeuronCores, SPMD):
  * Data-parallel pooling over seq_len, h and z sharded independently:
    h_pool is only consumed through h_pool[start:end] (353 live rows), so only
    those rows are streamed (45/core); all 512 z rows are live (64/core).
    z streams on the SP HWDGE ring, h on the ACT ring.
  * z is pooled first (PE block-matmuls against a 1/N vector) and its pooled
    means AllGathered early; the replicated z epilogue (projection, distance
    term, row-normalisation) is computed feature-major ([H, S]) so row norms
    are one ones-matmul column reduction - no transposes, no DRAM bounce.
  * The NCE is sharded over t_sample: each core scores only its own 45
    pooled-h timesteps (no h AllGather at all).  The per-core window of
    normalised z_pool columns is carved out of the replicated zhat with ONE
    dynamically-offset copy (offset register loaded from a per-core uint32
    input), after which all 14 shift slices are static.  Core 7's range is
    clipped to [END-45, END) and the 7 rows it shares with core 6 are zeroed
    via a per-core mask row baked into its consts input.
  * cosine sims feature-major: per shift one DVE multiply [H, 45] plus one
    ones-matmul column reduction into PSUM rows; log-softmax reduces to
    overlapping-window reductions on a [1, 630] row.
  * A final 8-byte AllReduce sums the per-core NCE partials (distance is
    replicated and just divided by NCORES).

The kernel function takes FULL unsharded inputs and returns the full output
tuple (nce_loss, distance), both float32 scalars.
"""

import os
import sys

import numpy as np

for _p in ("/opt/trn_rl_repo",):
    if _p not in sys.path and os.path.isdir(_p):
        sys.path.insert(0, _p)

import concourse.bacc as bacc
import concourse.bass as bass
import concourse.mybir as mybir
import concourse.tile as tile

F32 = mybir.dt.float32
U32 = mybir.dt.uint32
AX = mybir.AxisListType
OP = mybir.AluOpType
AF = mybir.ActivationFunctionType

# Problem constants (hardcoded; see module docstring).
S, N, H, Z = 512, 1024, 128, 64
NCORES = 8
NB = N // 128              # 8 node sub-blocks per SBUF partition
SAMPLE_NUM, TIMESPAN = 8, 4
EPS = 1e-8
NEG_DIST = S // 6          # 85
END = S - SAMPLE_NUM - NEG_DIST - TIMESPAN + 2    # 417
START = S // 8             # 64
CNT = END - START          # 353
SZ = S // NCORES           # 64 z timesteps per core
SH = 45                    # h timesteps per core (t-shard width)
HBATCHES = [9, 9, 9, 9, 3, 3, 3]   # h DMA batches (sum=45); small tail
HB = max(HBATCHES)
ZB = 8                     # z DMA batch (64 = 8*8) -> 2.0 MB per DMA
# shifts c = i + offs[m]; m=0 -> c=i (positives), m>=1 -> c=84+i+m in 86..95
SHIFTS = [1, 2, 3, 4] + list(range(86, 96))
NC14 = len(SHIFTS)         # 14
NPOS = TIMESPAN            # 4 positive shift blocks
NNEG = NC14 - NPOS         # 10 negative shift blocks
WWIN = SH + SHIFTS[-1] - 1  # 139: zhat cols [t0+1, t0+WWIN] cover all windows
TMAX = END - SH            # 372: largest per-core t0 (core 7, clipped)

# packed-constants column layout (one [128, CW] input)
_C_WZT = 0          # [0:64, 0:128]   Wz.T
_C_WH = 128         # [:, 128:256]    Wh
_C_WPHIT = 256      # [:, 256:384]    Wphi.T
_C_BZ = 384         # [:, 384]        bz
_C_BH = 385         # [:, 385]        bh
_C_WVEC = 386       # [:, 386]        1/N
_C_ONES = 387       # [:, 387]        ones column
_C_ONES_R = 388     # [0, 388:900]    ones row (512)
_C_BPHI = 900       # [0, 900:1028]   bphi row
_C_MASK = 1028      # [0, 1028:1073]  per-core t-validity mask row (45)
_C_SCL = 1073       # [0, 1073:1075]  [-1/(CNT*TIMESPAN), 1/(NCORES*S)]
CW = 1075


# timing-experiment switches (production = all False); set via profile scripts
_OPTS = {
    "streams_only": False,   # stop after pooling (wrong output; DMA phase only)
    "dma_only": False,       # with streams_only: skip pooling compute too
    "no_zag": False,         # skip z AllGather (wrong output; timing only)
    "no_ar": False,          # skip final AllReduce (wrong output; timing only)
    "finish": "ag",          # "ag": AllGather + local sum; "ar": AllReduce
    "h_reduce": "strided",   # "strided" | "tree" | "off" (timing only)
    "z_pool": "pe",          # "pe" | "off" (timing only)
    "h_matvec": True,        # False: skip h matvecs (timing only)
}


def _emit(nc, tc, aps, reps=1):
    for _ in range(reps):
        _emit_once(nc, tc, aps)


def _emit_once(nc, tc, aps):
    ah, az = aps["ah"], aps["az"]
    out = aps["out"]
    ag_groups = [list(range(NCORES))]

    with tc.tile_pool(name="const", bufs=1) as cpool, \
         tc.tile_pool(name="stream", bufs=2) as spool, \
         tc.tile_pool(name="work", bufs=2) as wpool, \
         tc.tile_pool(name="prod", bufs=4) as prodpool, \
         tc.tile_pool(name="dram", bufs=1, space="DRAM") as dpool, \
         tc.tile_pool(name="psumK", bufs=1, space="PSUM") as ppoolK:

        consts = cpool.tile([128, CW], F32, tag="consts")
        nc.sync.dma_start(consts[:], aps["consts"])
        wzt_sb = consts[0:Z, _C_WZT:_C_WZT + H]
        wh_sb = consts[:, _C_WH:_C_WH + H]
        wphit_sb = consts[:, _C_WPHIT:_C_WPHIT + H]
        bz_sb = consts[:, _C_BZ:_C_BZ + 1]
        bh_sb = consts[:, _C_BH:_C_BH + 1]
        wvec_sb = consts[:, _C_WVEC:_C_WVEC + 1]
        ones_sb = consts[:, _C_ONES:_C_ONES + 1]
        ones_r_sb = consts[0:1, _C_ONES_R:_C_ONES_R + S]
        bphi_sb = consts[0:1, _C_BPHI:_C_BPHI + H]
        mask_sb = consts[0:1, _C_MASK:_C_MASK + SH]
        scl_sb = consts[0:1, _C_SCL:_C_SCL + 2]

        # distance accumulator (written once by the dssq matmul)
        psum_dist = ppoolK.tile([1, 1], F32, tag="psum_dist")

        # fused context weights WcT = (Wphi @ Wh).T and bias bc = Wphi@bh+bphi
        with tc.tile_pool(name="psumW", bufs=1, space="PSUM") as ppoolW:
            psum_wct = ppoolW.tile([H, H], F32, tag="psum_wct")
            nc.tensor.matmul(psum_wct[:], wh_sb, wphit_sb,
                             start=True, stop=True, skip_group_check=True)
            wct_sb = wpool.tile([H, H], F32, tag="wct_sb")
            nc.scalar.copy(wct_sb[:], psum_wct[:])
            psum_bc = ppoolW.tile([1, H], F32, tag="psum_bc")
            nc.tensor.matmul(psum_bc[:], bh_sb, wphit_sb,
                             start=True, stop=False, skip_group_check=True)
            nc.tensor.matmul(psum_bc[:], ones_sb[0:1, 0:1], bphi_sb,
                             start=False, stop=True, skip_group_check=True)
            bc_sb = wpool.tile([1, H], F32, tag="bc_sb")
            nc.scalar.copy(bc_sb[:], psum_bc[:])

        # ------------- z pooling (PE) + early AllGather --------------------
        with tc.tile_pool(name="psumZ", bufs=1, space="PSUM") as ppoolZ:
            psum_zmT = ppoolZ.tile([Z, SZ], F32, tag="psum_zmT")   # [z, s]
            for g in range(SZ // ZB):
                zbuf = spool.tile([128, ZB * NB * Z], F32, tag="zbuf")
                nc.sync.dma_start(
                    zbuf[:].rearrange("p (b f) -> p b f", b=ZB),
                    az[g * ZB:(g + 1) * ZB].rearrange("b p f -> p b f"))
                if _OPTS["dma_only"] or _OPTS["z_pool"] == "off":
                    continue
                for k in range(ZB):
                    s = g * ZB + k
                    for nb in range(NB):
                        off = k * NB * Z + nb * Z
                        nc.tensor.matmul(
                            psum_zmT[:, s:s + 1],
                            zbuf[:, off:off + Z], wvec_sb,
                            start=(nb == 0), stop=(nb == NB - 1),
                            skip_group_check=True)
            zmT_sb = wpool.tile([Z, SZ], F32, tag="zmT_sb")
            if not (_OPTS["dma_only"] or _OPTS["z_pool"] == "off"):
                nc.scalar.copy(zmT_sb[:], psum_zmT[:])

        zmT = wpool.tile([Z, S], F32, tag="zmT")            # [z, s_global]
        if _OPTS["no_zag"]:
            if not (_OPTS["dma_only"] or _OPTS["z_pool"] == "off"):
                for c in range(NCORES):
                    nc.scalar.copy(zmT[:, c * SZ:(c + 1) * SZ], zmT_sb[:])
        else:
            cc_in_z = dpool.tile([1, Z * SZ], F32, tag="cc_in_z")
            nc.scalar.dma_start(
                cc_in_z[0, :].rearrange("(z s) -> z s", z=Z), zmT_sb[:])
            cc_out_z = dpool.tile([NCORES, Z * SZ], F32, tag="cc_out_z")
            nc.gpsimd.collective_compute(
                "AllGather", OP.bypass, replica_groups=ag_groups,
                ins=[cc_in_z[:].opt()], outs=[cc_out_z[:].opt()])
            nc.scalar.dma_start(
                zmT[:].rearrange("p (c s) -> p c s", c=NCORES),
                cc_out_z[:, :].rearrange("c (z s) -> z c s", z=Z))

        # ------------- h pooling (ACT-ring DMA + DVE + PE matvecs) ---------
        # Emitted before the z epilogue so the in-order DVE/ACT streams keep
        # the h DMAs flowing; the z epilogue below overlaps this phase.
        hmT_sb = wpool.tile([H, SH], F32, tag="hmT_sb")
        with tc.tile_pool(name="psumH", bufs=1, space="PSUM") as ppoolH:
            psum_hmT = ppoolH.tile([H, SH], F32, tag="psum_hmT")
            s0 = 0
            for hb in HBATCHES:
                hbuf = spool.tile([128, HB * NB * H], F32, tag="hbuf")
                nc.scalar.dma_start(
                    hbuf[:, 0:hb * NB * H].rearrange(
                        "p (b f) -> p b f", b=hb),
                    ah[s0:s0 + hb].rearrange("b p f -> p b f"))
                if _OPTS["dma_only"] or _OPTS["h_reduce"] == "off":
                    s0 += hb
                    continue
                hpart = wpool.tile([128, HB * H], F32, tag="hpart")
                if _OPTS["h_reduce"] == "tree":
                    # unit-stride pairwise tree over nb: 8 -> 4 -> 2 -> 1.
                    # All reads/writes are 128-contiguous runs, unlike the
                    # stride-128 gather of the plain reduce.
                    def _ap(t, off, dims):
                        b_ = t[:]
                        return bass.AP(b_.tensor, b_.offset + off,
                                       [[b_.ap[0][0], 128]] + dims)
                    t4 = wpool.tile([128, HB * 4 * H], F32, tag="htree4")
                    nc.vector.tensor_add(
                        _ap(t4, 0, [[4 * H, hb], [H, 4], [1, H]]),
                        _ap(hbuf, 0, [[NB * H, hb], [2 * H, 4], [1, H]]),
                        _ap(hbuf, H, [[NB * H, hb], [2 * H, 4], [1, H]]))
                    t2 = wpool.tile([128, HB * 2 * H], F32, tag="htree2")
                    nc.vector.tensor_add(
                        _ap(t2, 0, [[2 * H, hb], [H, 2], [1, H]]),
                        _ap(t4, 0, [[4 * H, hb], [2 * H, 2], [1, H]]),
                        _ap(t4, H, [[4 * H, hb], [2 * H, 2], [1, H]]))
                    nc.vector.tensor_add(
                        _ap(hpart, 0, [[H, hb], [1, H]]),
                        _ap(t2, 0, [[2 * H, hb], [1, H]]),
                        _ap(t2, H, [[2 * H, hb], [1, H]]))
                else:
                    nc.vector.reduce_sum(
                        hpart[:, 0:hb * H],
                        hbuf[:, 0:hb * NB * H].rearrange(
                            "p (b nb h) -> p b h nb", b=hb, nb=NB),
                        axis=AX.X)
                if _OPTS["h_matvec"]:
                    for k in range(hb):
                        s = s0 + k
                        nc.tensor.matmul(
                            psum_hmT[:, s:s + 1],
                            hpart[:, k * H:(k + 1) * H], wvec_sb,
                            start=True, stop=True, skip_group_check=True)
                s0 += hb
            if not (_OPTS["dma_only"] or _OPTS["h_reduce"] == "off"
                    or not _OPTS["h_matvec"]):
                nc.scalar.copy(hmT_sb[:], psum_hmT[:])

        if _OPTS["streams_only"]:
            h_off = (_OPTS["dma_only"] or _OPTS["h_reduce"] == "off"
                     or not _OPTS["h_matvec"])
            z_off = _OPTS["dma_only"] or _OPTS["z_pool"] == "off"
            out_sb0 = wpool.tile([1, 2], F32, tag="out_sb0")
            nc.scalar.copy(out_sb0[0:1, 0:1],
                           consts[0:1, 0:1] if h_off else hmT_sb[0:1, 0:1])
            nc.scalar.copy(out_sb0[0:1, 1:2],
                           consts[0:1, 1:2] if z_off else zmT[0:1, 0:1])
            nc.sync.dma_start(out[:], out_sb0[:])
            return

        # ------------- z epilogue (replicated; overlaps h pooling) ---------
        # Everything stays feature-major [H, S]: per-row z norms become
        # ones-matmul column reductions, no transposes.
        zhat = wpool.tile([H, S], F32, tag="zhat")
        zwin = wpool.tile([H, WWIN], F32, tag="zwin")
        with tc.tile_pool(name="psumE", bufs=1, space="PSUM") as ppoolE:
            psum_zp = ppoolE.tile([H, S], F32, tag="psum_zp")
            nc.tensor.matmul(psum_zp[:], wzt_sb, zmT[:],
                             start=True, stop=True, skip_group_check=True)
            zps = wpool.tile([H, S], F32, tag="zps")
            nc.scalar.activation(zps[:], psum_zp[:], AF.Identity,
                                 bias=bz_sb)

            # distance = sum((z_pool - gmean)^2) / S
            gsum = wpool.tile([H, 1], F32, tag="gsum")
            nc.vector.reduce_sum(gsum[:], zps[:], axis=AX.X)
            gmean = wpool.tile([H, 1], F32, tag="gmean")
            nc.scalar.mul(gmean[:], gsum[:], 1.0 / S)
            zc = wpool.tile([H, S], F32, tag="zc")
            nc.vector.tensor_scalar(out=zc[:], in0=zps[:], scalar1=gmean[:],
                                    scalar2=None, op0=OP.subtract)
            dsq = wpool.tile([H, S], F32, tag="dsq")
            dssq = wpool.tile([H, 1], F32, tag="dssq")
            nc.scalar.activation(dsq[:], zc[:], AF.Square, accum_out=dssq[:])
            nc.tensor.matmul(psum_dist[:], dssq[:], ones_sb,
                             start=True, stop=True, skip_group_check=True)

            # column norms -> zhat = zps * 1/max(||z_pool[s]||, eps)
            sqz = wpool.tile([H, S], F32, tag="sqz")
            nc.scalar.activation(sqz[:], zps[:], AF.Square)
            psum_zn = ppoolE.tile([1, S], F32, tag="psum_zn")
            nc.tensor.matmul(psum_zn[:], ones_sb, sqz[:],
                             start=True, stop=True, skip_group_check=True)
            nrmz = wpool.tile([1, S], F32, tag="nrmz")
            nc.scalar.sqrt(nrmz[:], psum_zn[:])
            nc.vector.tensor_scalar_max(nrmz[:], nrmz[:], EPS)
            rz = wpool.tile([1, S], F32, tag="rz")
            nc.vector.reciprocal(rz[:], nrmz[:])
            psum_rzb = ppoolE.tile([H, S], F32, tag="psum_rzb")
            nc.tensor.matmul(psum_rzb[:], ones_r_sb[0:1, 0:H], rz[:],
                             start=True, stop=True, skip_group_check=True)
            rzb = wpool.tile([H, S], F32, tag="rzb")
            nc.scalar.copy(rzb[:], psum_rzb[:])
            nc.vector.tensor_tensor(out=zhat[:], in0=zps[:], in1=rzb[:],
                                    op=OP.mult)

            # per-core window of zhat columns [t0+1, t0+WWIN]; t0 comes from
            # the per-core uint32 input, so one dynamic slice covers all 14
            # static shift slices below.
            treg = nc.scalar.alloc_register("t0_reg%d" % nc.next_id())
            nc.scalar.reg_load(treg, aps["toff"][0:1, 0:1])
            tval = nc.scalar.snap(treg, donate=True, min_val=0, max_val=TMAX)
            nc.scalar.copy(zwin[:], zhat[:, 1:S][:, bass.ds(tval, WWIN)])

        # ------------- NCE (t-sharded; local h only) -----------------------
        with tc.tile_pool(name="psumC", bufs=1, space="PSUM") as ppoolC:
            # context projection, feature-major [H, SH]
            psum_cph = ppoolC.tile([H, SH], F32, tag="psum_cph")
            nc.tensor.matmul(psum_cph[:], wct_sb[:], hmT_sb[:],
                             start=True, stop=False, skip_group_check=True)
            nc.tensor.matmul(psum_cph[:], bc_sb[:], ones_r_sb[0:1, 0:SH],
                             start=False, stop=True, skip_group_check=True)

            # column norms -> cphihat = c_phi * 1/max(||c_phi[t]||, eps)
            sqc = wpool.tile([H, SH], F32, tag="sqc")
            nc.scalar.activation(sqc[:], psum_cph[:], AF.Square)
            psum_cn = ppoolC.tile([1, SH], F32, tag="psum_cn")
            nc.tensor.matmul(psum_cn[:], ones_sb, sqc[:],
                             start=True, stop=True, skip_group_check=True)
            nrmc = wpool.tile([1, SH], F32, tag="nrmc")
            nc.scalar.sqrt(nrmc[:], psum_cn[:])
            nc.vector.tensor_scalar_max(nrmc[:], nrmc[:], EPS)
            rc = wpool.tile([1, SH], F32, tag="rc")
            nc.vector.reciprocal(rc[:], nrmc[:])
            psum_rcb = ppoolC.tile([H, SH], F32, tag="psum_rcb")
            nc.tensor.matmul(psum_rcb[:], ones_r_sb[0:1, 0:H], rc[:],
                             start=True, stop=True, skip_group_check=True)
            rcb = wpool.tile([H, SH], F32, tag="rcb")
            nc.scalar.copy(rcb[:], psum_rcb[:])
            cph = wpool.tile([H, SH], F32, tag="cph")
            nc.vector.tensor_tensor(out=cph[:], in0=psum_cph[:], in1=rcb[:],
                                    op=OP.mult)

            # cosine sims: per shift one DVE multiply + one ones-matmul
            # column reduction; results land as [1, 45] blocks in PSUM.
            psum_dp = ppoolC.tile([1, NPOS * SH], F32, tag="psum_dp")
            psum_dn = ppoolC.tile([1, NNEG * SH], F32, tag="psum_dn")
            for j, delta in enumerate(SHIFTS):
                prod = prodpool.tile([H, SH], F32, tag="prod")
                nc.vector.tensor_tensor(
                    out=prod[:], in0=zwin[:, delta - 1:delta - 1 + SH],
                    in1=cph[:], op=OP.mult)
                dst = (psum_dp[:, j * SH:(j + 1) * SH] if j < NPOS
                       else psum_dn[:, (j - NPOS) * SH:(j - NPOS + 1) * SH])
                nc.tensor.matmul(dst, ones_sb, prod[:],
                                 start=True, stop=True, skip_group_check=True)

            # log-softmax over the 8 samples; positive at m=0
            expd = wpool.tile([1, NC14 * SH], F32, tag="expd")
            nc.scalar.activation(expd[0:1, 0:NPOS * SH], psum_dp[:], AF.Exp)
            nc.scalar.activation(expd[0:1, NPOS * SH:NC14 * SH], psum_dn[:],
                                 AF.Exp)
            den = wpool.tile([1, NPOS * SH], F32, tag="den")
            eb = expd[:]
            for ii in range(TIMESPAN):
                # negatives for step ii live in 7 consecutive 45-blocks
                neg_ap = bass.AP(
                    eb.tensor, eb.offset + (NPOS + ii) * SH,
                    [[eb.ap[0][0], 1], [1, SH], [SH, 7]])
                nc.vector.reduce_sum(den[0:1, ii * SH:(ii + 1) * SH],
                                     neg_ap, axis=AX.X)
            nc.vector.tensor_add(den[:], den[:], expd[0:1, 0:NPOS * SH])
            lse = wpool.tile([1, NPOS * SH], F32, tag="lse")
            nc.scalar.activation(lse[:], den[:], AF.Ln)
            ctr = wpool.tile([1, NPOS * SH], F32, tag="ctr")
            nc.vector.tensor_sub(ctr[:], psum_dp[:], lse[:])
            cb = ctr[:]
            ctrt = wpool.tile([1, SH], F32, tag="ctrt")
            sum_ap = bass.AP(cb.tensor, cb.offset,
                             [[cb.ap[0][0], 1], [1, SH], [SH, TIMESPAN]])
            nc.vector.reduce_sum(ctrt[:], sum_ap, axis=AX.X)
            masked = wpool.tile([1, SH], F32, tag="masked")
            nc.vector.tensor_tensor(out=masked[:], in0=ctrt[:], in1=mask_sb,
                                    op=OP.mult)
            nce1 = wpool.tile([1, 1], F32, tag="nce1")
            nc.vector.reduce_sum(nce1[:], masked[:], axis=AX.X)

        # ------------- final scalar AllReduce + scale ----------------------
        stage_sb = wpool.tile([1, 2], F32, tag="stage_sb")
        nc.vector.tensor_copy(out=stage_sb[0:1, 0:1], in_=nce1[:])
        nc.scalar.copy(stage_sb[0:1, 1:2], psum_dist[:])
        out_sb = wpool.tile([1, 2], F32, tag="out_sb")
        if _OPTS["no_ar"]:
            nc.vector.tensor_tensor(out=out_sb[:], in0=stage_sb[:],
                                    in1=scl_sb, op=OP.mult)
            nc.sync.dma_start(out[:], out_sb[:])
            return
        cc_in_s = dpool.tile([1, 2], F32, tag="cc_in_s")
        nc.sync.dma_start(cc_in_s[:], stage_sb[:])
        if _OPTS["finish"] == "ag":
            cc_out_s = dpool.tile([NCORES, 2], F32, tag="cc_out_s")
            nc.gpsimd.collective_compute(
                "AllGather", OP.bypass, replica_groups=ag_groups,
                ins=[cc_in_s[:].opt()], outs=[cc_out_s[:].opt()])
            ag_sb = wpool.tile([NCORES, 2], F32, tag="ag_sb")
            nc.sync.dma_start(ag_sb[:], cc_out_s[:])
            with tc.tile_pool(name="psumF", bufs=1, space="PSUM") as ppoolF:
                psum_f = ppoolF.tile([1, 2], F32, tag="psum_f")
                nc.tensor.matmul(psum_f[:], ones_sb[0:NCORES, 0:1], ag_sb[:],
                                 start=True, stop=True,
                                 skip_group_check=True)
                nc.vector.tensor_tensor(out=out_sb[:], in0=psum_f[:],
                                        in1=scl_sb, op=OP.mult)
        else:
            cc_out_s = dpool.tile([1, 2], F32, tag="cc_out_s")
            nc.gpsimd.collective_compute(
                "AllReduce", OP.add, replica_groups=ag_groups,
                ins=[cc_in_s[:].opt()], outs=[cc_out_s[:].opt()])
            ar_sb = wpool.tile([1, 2], F32, tag="ar_sb")
            nc.sync.dma_start(ar_sb[:], cc_out_s[:])
            nc.vector.tensor_tensor(out=out_sb[:], in0=ar_sb[:], in1=scl_sb,
                                    op=OP.mult)
        nc.sync.dma_start(out[:], out_sb[:])


def _build(reps=1):
    nc = bacc.Bacc("TRN2", debug=False, enable_asserts=False,
                   target_bir_lowering=False, num_devices=NCORES)
    aps = {}

    def din(name, shape, dt=F32):
        aps[name] = nc.dram_tensor(name, shape, dt, kind="ExternalInput").ap()

    din("ah", [SH, 128, NB * H])
    din("az", [SZ, 128, NB * Z])
    din("consts", [128, CW])
    din("toff", [1, 1], U32)
    aps["out"] = nc.dram_tensor("out", [1, 2], F32,
                                kind="ExternalOutput").ap()

    with tile.TileContext(nc) as tc:
        _emit(nc, tc, aps, reps=reps)
    nc.compile()
    return nc


_CACHE = {}


def _core_t0(c):
    """Global first t_sample of core c's shard (core 7 clipped to END-45)."""
    return min(START + SH * c, END - SH)


def _pack_consts(Wh, bh, Wz, bz, Wphi, bphi, core):
    c = np.zeros((128, CW), dtype=np.float32)
    c[0:Z, _C_WZT:_C_WZT + H] = Wz.T
    c[:, _C_WH:_C_WH + H] = Wh
    c[:, _C_WPHIT:_C_WPHIT + H] = Wphi.T
    c[:, _C_BZ] = bz
    c[:, _C_BH] = bh
    c[:, _C_WVEC] = 1.0 / N
    c[:, _C_ONES] = 1.0
    c[0, _C_ONES_R:_C_ONES_R + S] = 1.0
    c[0, _C_BPHI:_C_BPHI + H] = bphi
    # rows whose global index is owned by a lower core are masked off
    # (only core 7's clipped shard overlaps core 6's)
    t0 = _core_t0(core)
    m = (np.arange(t0, t0 + SH) >= START + SH * core).astype(np.float32)
    c[0, _C_MASK:_C_MASK + SH] = m
    c[0, _C_SCL] = -1.0 / (CNT * TIMESPAN)
    c[0, _C_SCL + 1] = 1.0 / (NCORES * S)
    return c


def make_in_maps(all_h, all_z, Wh, bh, Wz, bz, Wphi, bphi):
    in_maps = []
    for c in range(NCORES):
        t0 = _core_t0(c)
        in_maps.append({
            "consts": _pack_consts(Wh, bh, Wz, bz, Wphi, bphi, c),
            "toff": np.array([[t0]], dtype=np.uint32),
            "ah": np.ascontiguousarray(
                all_h[t0:t0 + SH].reshape(SH, 128, NB * H)),
            "az": np.ascontiguousarray(
                all_z[c * SZ:(c + 1) * SZ].reshape(SZ, 128, NB * Z)),
        })
    return in_maps


def _get_runner():
    """Build the Bass program and one jitted shard_map executable, once.

    Re-lowering a fresh executable per call reloads the collective NEFF and
    leaves NRT unrecoverable on the second call, so the executable is cached
    and every kernel() invocation reuses it with freshly uploaded inputs.
    """
    if "runner" in _CACHE:
        return _CACHE["runner"]

    import jax
    from concourse import bass2jax
    from concourse.bass2jax import _bass_exec_p, partition_id_tensor
    from jax.sharding import Mesh, PartitionSpec, NamedSharding
    from jax.experimental.shard_map import shard_map

    nc = _build()
    bass2jax.install_neuronx_cc_hook()
    partition_name = (nc.partition_id_tensor.name
                      if nc.partition_id_tensor else None)

    in_names, out_names, out_avals, zero_outs = [], [], [], []
    for alloc in nc.m.functions[0].allocations:
        if not isinstance(alloc, mybir.MemoryLocationSet):
            continue
        name = alloc.memorylocations[0].name
        if alloc.kind == "ExternalInput":
            if name != partition_name:
                in_names.append(name)
        elif alloc.kind == "ExternalOutput":
            shape = tuple(alloc.tensor_shape)
            dtype = mybir.dt.np(alloc.dtype)
            out_names.append(name)
            out_avals.append(jax.core.ShapedArray(shape, dtype))
            zero_outs.append(np.zeros(shape, dtype))
    n_params = len(in_names)
    all_in_names = list(in_names) + out_names
    if partition_name is not None:
        all_in_names.append(partition_name)

    def _body(*args):
        operands = list(args)
        if partition_name is not None:
            operands.append(partition_id_tensor())
        outs = _bass_exec_p.bind(
            *operands,
            out_avals=tuple(out_avals),
            in_names=tuple(all_in_names),
            out_names=tuple(out_names),
            lowering_input_output_aliases=(),
            sim_require_finite=True,
            sim_require_nnan=True,
            nc=nc,
        )
        return tuple(outs)

    devices = jax.devices()[:NCORES]
    mesh = Mesh(np.asarray(devices), ("core",))
    n_outs = len(out_avals)
    in_specs = (PartitionSpec("core"),) * (n_params + n_outs)
    out_specs = (PartitionSpec("core"),) * n_outs
    sharded = jax.jit(shard_map(_body, mesh=mesh, in_specs=in_specs,
                                out_specs=out_specs, check_rep=False),
                      keep_unused=True)
    sh = NamedSharding(mesh, PartitionSpec("core"))
    dev_zeros = [
        jax.device_put(
            np.zeros((NCORES * z.shape[0], *z.shape[1:]), z.dtype), sh)
        for z in zero_outs
    ]

    def run(in_maps):
        dev_in = [
            jax.device_put(
                np.concatenate([np.asarray(in_maps[c][n])
                                for c in range(NCORES)], axis=0), sh)
            for n in in_names
        ]
        outs = sharded(*dev_in, *dev_zeros)
        return [
            {name: np.asarray(outs[i]).reshape(NCORES, *out_avals[i].shape)[c]
             for i, name in enumerate(out_names)}
            for c in range(NCORES)
        ]

    _CACHE["runner"] = run
    return run


def kernel(all_h, all_z, Wh, bh, Wz, bz, Wphi, bphi):
    all_h = np.ascontiguousarray(np.asarray(all_h, dtype=np.float32))
    all_z = np.ascontiguousarray(np.asarray(all_z, dtype=np.float32))
    args = [np.asarray(x, dtype=np.float32)
            for x in (Wh, bh, Wz, bz, Wphi, bphi)]

    # The axon NTFF trace hook (antenv.axon_hooks) is absent in this image;
    # make sure an inherited BASS_TRACE can't route us onto that path.
    os.environ["BASS_NEVER_TRACE"] = "1"

    run = _get_runner()
    in_maps = make_in_maps(all_h, all_z, *args)
    results = run(in_maps)
    _CACHE["last_results"] = results

    o = results[0]["out"]
    nce_loss = np.asarray(o[0, 0], dtype=np.float32)
    distance = np.asarray(o[0, 1], dtype=np.float32)
    return (nce_loss, distance)

